# revision 25
# baseline (speedup 1.0000x reference)
"""Graphormer forward on 8 Trainium2 NeuronCores (Bass/Tile).

B=32, N=512, D=256, H=8, L=2. Data-parallel over B, strided: core k owns
graphs {k, k+8, k+16, k+24}. Host stages inputs (transposes x, computes the
degree-embedding addend and the exact SPD attention-bias slabs in bf16,
packs weights); the device does all matmuls (f32r; P*V in bf16), softmax,
layer-norms, and both BatchNorms (stats all-reduced across the 8 cores with
on-chip collectives).

Everything runs in transposed layout hT[D, tokens] so each matmul finds its
contraction on the partition axis without runtime transposes: attention
scores as sT[j, i]; softmax denominators from a ones-column appended to V
(one matmul emits o^T and the denominator); per-head 1/denom broadcast with
a K=1 ones matmul; LN stats via ones-vector matmuls; BN stats via
free-axis vector reductions.
"""

import os
import sys
import time

os.environ.setdefault("JAX_PLATFORMS", "")
if "/opt/trn_rl_repo" not in sys.path:
    sys.path.insert(0, "/opt/trn_rl_repo")

import numpy as np

import concourse.bass as bass
import concourse.bacc as bacc
import concourse.mybir as mybir
import concourse.tile as tile
from concourse.bass_utils import run_bass_kernel_spmd

f32 = mybir.dt.float32
f32r = mybir.dt.float32r
bf16 = mybir.dt.bfloat16
AF = mybir.ActivationFunctionType
ALU = mybir.AluOpType

B, N, DIN, D, H, L, DOUT = 32, 512, 256, 256, 8, 2, 256
DH = D // H
G = 4                  # graphs per core
NI = G * N             # tokens per core (2048)
NC = 8                 # cores
EPS = 1e-5
SCALE = 1.0 / np.sqrt(DH)

OFF_WF = 0
LBLK = 512 + 264 + 256 + 256 + 256      # qk, v_aug, wo, w1, w2
OFF_L = [256 + l * LBLK for l in range(L)]
OFF_WIN = 256 + L * LBLK
WCOLS = OFF_WIN + 256                   # 3600
RCOLS = WCOLS + 512                     # + ones row

LAST_EXEC_NS = None
_CACHE = {}


def _bf16(x):
    u = np.asarray(x, np.float32).view(np.uint32)
    r = ((u >> 16) & 1) + 0x7FFF
    return ((u + r) >> 16).astype(np.uint16)


def _patch_act_tables():
    """Force all in-loop activations (Exp/Ln/Relu/Square) into the single
    natural_log_exp_and_others set so the ACT engine never thrashes table
    loads; Lrelu keeps its own set (entry/exit only)."""
    import concourse.hw_specs as hw_specs
    if getattr(hw_specs, "_graphormer_patched", False):
        return
    orig = hw_specs.get_activation_tables

    def patched(arch):
        tabs = orig(arch)
        out = {}
        for name, funcs in tabs.items():
            if name == "natural_log_exp_and_others":
                out[name] = set(funcs)
            elif name == "derivative_gelu_apprx_sigmoid_and_others":
                out[name] = {AF.Lrelu} & set(funcs)
            else:
                out[name] = set()
        return out

    hw_specs.get_activation_tables = patched
    bacc.get_activation_tables = patched
    hw_specs._graphormer_patched = True


def build_nc():
    _patch_act_tables()
    nc = bacc.Bacc("TRN2", target_bir_lowering=False)

    xT_d = nc.dram_tensor("xT", [DIN, NI], f32, kind="ExternalInput")
    degT_d = nc.dram_tensor("degT", [D, NI], f32, kind="ExternalInput")
    bias_d = nc.dram_tensor("biasT", [G * H, 4, 128, N], bf16,
                            kind="ExternalInput")
    wpack_d = nc.dram_tensor("wpack", [D, WCOLS], f32, kind="ExternalInput")
    rows_d = nc.dram_tensor("rows", [1, RCOLS], f32, kind="ExternalInput")
    cols_d = nc.dram_tensor("cols", [128, 52], f32, kind="ExternalInput")
    ident_d = nc.dram_tensor("ident", [128, 128], f32, kind="ExternalInput")
    sel4_d = nc.dram_tensor("sel4", [128, 128], f32, kind="ExternalInput")
    out_d = nc.dram_tensor("out", [NI, DOUT], f32, kind="ExternalOutput")

    cc_in = [nc.dram_tensor(f"cc_in{i}", [128, 8], f32) for i in range(2)]
    cc_out = [
        nc.dram_tensor(f"cc_out{i}", [128, 8], f32, addr_space="Shared")
        for i in range(2)
    ]

    with tile.TileContext(nc) as tc:
        _build_body(nc, tc, xT_d, degT_d, bias_d, wpack_d, rows_d, cols_d,
                    ident_d, sel4_d, out_d, cc_in, cc_out)
    nc.compile()
    return nc


def _build_body(nc, tc, xT_d, degT_d, bias_d, wpack_d, rows_d, cols_d,
                ident_d, sel4_d, out_d, cc_in, cc_out):
    from contextlib import ExitStack

    ctx = ExitStack()
    wts = ctx.enter_context(tc.tile_pool(name="wts", bufs=1))
    big = ctx.enter_context(tc.tile_pool(name="big", bufs=1))
    work = ctx.enter_context(tc.tile_pool(name="work", bufs=1))
    ptp = ctx.enter_context(tc.tile_pool(name="ptp", bufs=2))
    otp = ctx.enter_context(tc.tile_pool(name="otp", bufs=1))
    rowp = ctx.enter_context(tc.tile_pool(name="rowp", bufs=1))
    colp = ctx.enter_context(tc.tile_pool(name="colp", bufs=2))
    biasp = ctx.enter_context(tc.tile_pool(name="biasp", bufs=2))
    outp = ctx.enter_context(tc.tile_pool(name="outp", bufs=2))
    ps = ctx.enter_context(tc.tile_pool(name="ps", bufs=2, space="PSUM"))
    ps1 = ctx.enter_context(tc.tile_pool(name="ps1", bufs=1, space="PSUM"))
    ps2 = ctx.enter_context(tc.tile_pool(name="ps2", bufs=2, space="PSUM"))

    # ---------- constants (DMA-cast f32 -> f32r on gpsimd queues) ----------
    rows_r = wts.tile([1, RCOLS], f32r, tag="rows_r", name="rows_r")
    nc.gpsimd.dma_start(rows_r[:], rows_d[:])
    colpk = wts.tile([128, 52], f32, tag="colpk", name="colpk")
    nc.sync.dma_start(colpk[:], cols_d[:])
    ident = wts.tile([128, 128], f32, tag="ident", name="ident")
    nc.sync.dma_start(ident[:], ident_d[:])
    ones_col = wts.tile([128, 1], f32r, tag="ones_col", name="ones_col")
    nc.vector.tensor_copy(ones_col[:], colpk[:, 24:25])
    sel4 = wts.tile([128, 128], f32r, tag="sel4", name="sel4")
    nc.gpsimd.dma_start(sel4[:], sel4_d[:])
    den32 = []
    for i in range(2):
        t = wts.tile([128, 512], f32, tag=f"den32_{i}", name=f"den32_{i}")
        nc.gpsimd.memset(t[:], 1.0)
        den32.append(t)
    wpr = []
    for c in range(2):
        t = wts.tile([128, WCOLS], f32r, tag=f"wpr{c}", name=f"wpr{c}")
        nc.gpsimd.dma_start(t[:], wpack_d[128 * c : 128 * c + 128, :])
        wpr.append(t)
    ones_row = rows_r[0:1, WCOLS : WCOLS + 512]

    degT = []
    for c in range(2):
        t = big.tile([128, NI], f32, tag=f"deg{c}", name=f"deg{c}")
        nc.sync.dma_start(t[:], degT_d[128 * c : 128 * c + 128, :])
        degT.append(t)

    # ---------- entry: yT = Wf @ xT + bf ----------
    yT = [big.tile([128, NI], f32, tag=f"ybuf{c}", name=f"ybuf{c}")
          for c in range(2)]
    with tc.tile_pool(name="xpool", bufs=1) as xpool:
        xr = []
        for c in range(2):
            t = xpool.tile([128, NI], f32r, tag=f"xr{c}", name=f"xr{c}")
            nc.gpsimd.dma_start(t[:], xT_d[128 * c : 128 * c + 128, :])
            xr.append(t)
        for et in range(2):
            for it in range(4):
                p = ps.tile([128, 512], f32, tag="mm", name="mm")
                for c in range(2):
                    nc.tensor.matmul(
                        p[:],
                        wpr[c][:, OFF_WF + 128 * et : OFF_WF + 128 * et + 128],
                        xr[c][:, 512 * it : 512 * it + 512],
                        start=(c == 0), stop=(c == 1))
                nc.vector.tensor_scalar(
                    yT[et][:, 512 * it : 512 * it + 512], p[:],
                    colpk[:, 28 + et : 29 + et], None, ALU.add)

    # ---------- BN1 + leaky + deg ----------
    h0 = _bn_leaky(nc, yT, colp, rowp, big, colpk, 0, cc_in[0], cc_out[0],
                   degT)

    # ---------- transformer layers ----------
    hcur = h0
    for l in range(L):
        hnew = [big.tile([128, NI], f32r, tag=f"h{(l + 1) % 2}_{c}",
                         name=f"h{(l + 1) % 2}_{c}") for c in range(2)]
        for g in range(G):
            _layer_graph(nc, l, g, hcur, hnew, wpr, rows_r, ones_row,
                         ones_col, colpk, bias_d, work, ptp, otp, rowp,
                         biasp, ps, ps1, ps2, sel4, den32)
        hcur = hnew

    # ---------- exit: y2 = W_in @ h + b_in ----------
    y2 = [big.tile([128, NI], f32, tag=f"ybuf{c}", name=f"y2_{c}")
          for c in range(2)]
    for et in range(2):
        for it in range(4):
            p = ps.tile([128, 512], f32, tag="mm", name="mm")
            for c in range(2):
                nc.tensor.matmul(
                    p[:],
                    wpr[c][:, OFF_WIN + 128 * et : OFF_WIN + 128 * et + 128],
                    hcur[c][:, 512 * it : 512 * it + 512],
                    start=(c == 0), stop=(c == 1))
            nc.vector.tensor_scalar(
                y2[et][:, 512 * it : 512 * it + 512], p[:],
                colpk[:, 30 + et : 31 + et], None, ALU.add)

    outT = _bn_leaky(nc, y2, colp, rowp, big, colpk, 1, cc_in[1], cc_out[1],
                     None)

    # ---------- transpose [256, 2048] -> [2048, 256], store ----------
    for r in range(16):
        o = outp.tile([128, 256], f32, tag="os", name="os")
        for c in range(2):
            pt = ps.tile([128, 512], f32, tag="mm", name="mmT")
            nc.tensor.transpose(
                pt[:, 0:128], outT[c][:, 128 * r : 128 * r + 128], ident[:])
            nc.vector.tensor_copy(o[:, 128 * c : 128 * c + 128], pt[:, 0:128])
        nc.sync.dma_start(out_d[128 * r : 128 * r + 128, :], o[:])

    ctx.close()


def _bn_leaky(nc, yT, colp, rowp, big, colpk, which, cc_in_d, cc_out_d,
              degT):
    """BatchNorm over all 16384 tokens (free-axis sums + 8-core allreduce),
    then leaky relu (+ deg addend at entry)."""
    gcol_off = 0 if which == 0 else 20
    pack = colp.tile([128, 8], f32, tag="ccpack", name="ccpack")
    for et in range(2):
        s = colp.tile([128, 1], f32, tag=f"sum{et}", name=f"sum{et}")
        nc.vector.reduce_sum(s[:], yT[et][:], axis=mybir.AxisListType.X)
        nc.vector.tensor_copy(pack[:, et : et + 1], s[:])
        sq = colp.tile([128, 4], f32, tag=f"sq{et}", name=f"sq{et}")
        for it in range(4):
            junk = colp.tile([128, 512], f32, tag="junk", name="junk")
            nc.scalar.activation(
                junk[:], yT[et][:, 512 * it : 512 * it + 512], AF.Square,
                accum_out=sq[:, it : it + 1])
        nc.vector.tensor_tensor(sq[:, 0:2], sq[:, 0:2], sq[:, 2:4], ALU.add)
        nc.vector.tensor_tensor(
            pack[:, 2 + et : 3 + et], sq[:, 0:1], sq[:, 1:2], ALU.add)
        nc.vector.tensor_copy(pack[:, 4 + 2 * et : 6 + 2 * et],
                              pack[:, 2 * et : 2 * et + 2])
    nc.sync.dma_start(cc_in_d[:], pack[:])
    nc.gpsimd.collective_compute(
        "AllReduce", ALU.add,
        ins=[cc_in_d[:]], outs=[cc_out_d[:]],
        replica_groups=[list(range(NC))])
    allst = colp.tile([128, 8], f32, tag="ccback", name="ccback")
    nc.sync.dma_start(allst[:], cc_out_d[:])

    out = []
    inv_n = 1.0 / (NC * NI)
    for et in range(2):
        mean = colp.tile([128, 1], f32, tag=f"mean{et}", name=f"mean{et}")
        nc.vector.tensor_scalar(mean[:], allst[:, et : et + 1], inv_n, None,
                                ALU.mult)
        esq = colp.tile([128, 1], f32, tag=f"esq{et}", name=f"esq{et}")
        nc.vector.tensor_scalar(esq[:], allst[:, 2 + et : 3 + et], inv_n,
                                None, ALU.mult)
        msq = colp.tile([128, 1], f32, tag=f"msq{et}", name=f"msq{et}")
        nc.vector.tensor_tensor(msq[:], mean[:], mean[:], ALU.mult)
        var = colp.tile([128, 1], f32, tag=f"var{et}", name=f"var{et}")
        nc.vector.tensor_tensor(var[:], esq[:], msq[:], ALU.subtract)
        lnv = colp.tile([128, 1], f32, tag=f"std{et}", name=f"lnv{et}")
        nc.scalar.activation(lnv[:], var[:], AF.Ln, bias=colpk[:, 25:26])
        inv = colp.tile([128, 1], f32, tag=f"inv{et}", name=f"inv{et}")
        nc.scalar.activation(inv[:], lnv[:], AF.Exp, scale=colpk[:, 27:28])
        colC = colp.tile([128, 1], f32, tag=f"colC{et}", name=f"colC{et}")
        nc.vector.tensor_tensor(
            colC[:], inv[:], colpk[:, gcol_off + et : gcol_off + et + 1],
            ALU.mult)
        colD = colp.tile([128, 1], f32, tag=f"colD{et}", name=f"colD{et}")
        nc.vector.tensor_tensor(colD[:], mean[:], colC[:], ALU.mult)
        nc.vector.tensor_tensor(
            colD[:], colpk[:, gcol_off + 2 + et : gcol_off + 3 + et],
            colD[:], ALU.subtract)
        if degT is not None:
            h = big.tile([128, NI], f32r, tag=f"h0_{et}", name=f"h0_{et}")
            nc.scalar.activation(h[:], yT[et][:], AF.Lrelu, bias=colD[:],
                                 scale=colC[:], alpha=0.01)
            nc.vector.tensor_tensor(h[:], h[:], degT[et][:], ALU.add)
            out.append(h)
        else:
            lk = big.tile([128, NI], f32, tag=f"deg{et}", name=f"lk{et}")
            nc.scalar.activation(lk[:], yT[et][:], AF.Lrelu, bias=colD[:],
                                 scale=colC[:], alpha=0.01)
            out.append(lk)
    return out


def _layer_graph(nc, l, g, hcur, hnew, wpr, rows_r, ones_row, ones_col,
                 colpk, bias_d, work, ptp, otp, rowp, biasp, ps, ps1, ps2,
                 sel4, den32):
    OQK = OFF_L[l]
    OV = OQK + 512
    OWO = OV + 264
    OW1 = OWO + 256
    OW2 = OW1 + 256
    hg = [hcur[c][:, 512 * g : 512 * g + 512] for c in range(2)]

    # ---- qkT as 8 head-pair tiles [64, 512] (q pairs 0-3, k pairs 4-7) ----
    qq = [work.tile([64, 512], f32r, tag=f"qq{i}", name=f"qq{i}")
          for i in range(8)]
    for m in range(4):
        p = ps.tile([128, 512], f32, tag="mm", name="mmqk")
        for c in range(2):
            nc.tensor.matmul(
                p[:], wpr[c][:, OQK + 128 * m : OQK + 128 * m + 128], hg[c],
                start=(c == 0), stop=(c == 1))
        bq = 32 + 10 * l + m
        nc.vector.tensor_scalar(qq[2 * m][:], p[0:64, :],
                                colpk[0:64, bq : bq + 1], None, ALU.add)
        nc.vector.tensor_scalar(qq[2 * m + 1][:], p[64:128, :],
                                colpk[64:128, bq : bq + 1], None, ALU.add)

    # ---- v_aug [512, 264] bf16, 4 j-tiles ----
    vb = []
    for jt in range(4):
        p = ps.tile([128, 512], f32, tag="mm", name="mmv")
        for c in range(2):
            nc.tensor.matmul(
                p[:, 0:264],
                hg[c][:, 128 * jt : 128 * jt + 128],
                wpr[c][:, OV : OV + 264],
                start=(c == 0), stop=False)
        nc.tensor.matmul(
            p[:, 0:264], ones_row[0:1, 0:128], rows_r[0:1, OV : OV + 264],
            start=False, stop=True)
        t = work.tile([128, 264], bf16, tag=f"vb{jt}", name=f"vb{jt}")
        nc.vector.tensor_copy(t[:], p[:, 0:264])
        vb.append(t)

    # ---- attention; o^T in 2 tiles [128, 512] (4 heads each), with
    # per-4-head-group deferred softmax normalization ----
    ot4 = [work.tile([128, 512], f32r, tag=f"ot4_{i}", name=f"ot4_{i}")
           for i in range(2)]
    for h in range(8):
        gi = h // 4
        dn = den32[gi]
        slab = biasp.tile([128, 2048], bf16, tag="bias", name="bias")
        src = bias_d[g * 8 + h].rearrange("jt p i -> p jt i")
        nc.sync.dma_start(slab[:].rearrange("p (jt i) -> p jt i", jt=4), src)

        qh = qq[h // 2][32 * (h % 2) : 32 * (h % 2) + 32, :]
        kt = qq[4 + h // 2][32 * (h % 2) : 32 * (h % 2) + 32, :]
        po = ps1.tile([128, 512], f32, tag="po", name="po", bufs=2)
        pt = ptp.tile([128, 2048], bf16, tag="pt", name="pt")
        for jt in range(4):
            p = ps2.tile([128, 512], f32, tag="ps_s", name="ps_s")
            nc.tensor.matmul(p[:], kt[:, 128 * jt : 128 * jt + 128], qh,
                             start=True, stop=True)
            nc.scalar.activation(pt[:, 512 * jt : 512 * jt + 512], p[:],
                                 AF.Exp)
        nc.vector.tensor_tensor(pt[:], pt[:], slab[:], ALU.mult)
        for jt in range(4):
            nc.tensor.matmul(
                po[0:33, :], vb[jt][:, 33 * h : 33 * h + 33],
                pt[:, 512 * jt : 512 * jt + 512],
                start=(jt == 0), stop=(jt == 3))
        lnr = rowp.tile([1, 512], f32, tag="recf", name="lnr")
        nc.scalar.activation(lnr[:], po[32:33, :], AF.Ln)
        rec = rowp.tile([1, 512], f32r, tag="rec", name="rec")
        nc.scalar.activation(rec[:], lnr[:], AF.Exp,
                             scale=colpk[0:1, 26:27])
        pb = ps2.tile([128, 512], f32, tag="ps_s", name="pb")
        nc.tensor.matmul(pb[0:32, :], ones_row[0:1, 0:32], rec[:],
                         start=True, stop=True)
        rsb = otp.tile([32, 512], f32, tag="rsb", name="rsb")
        nc.vector.tensor_copy(rsb[:], pb[0:32, :])
        dst = ot4[gi][32 * (h % 4) : 32 * (h % 4) + 32, :]
        nc.vector.tensor_tensor(dst, po[0:32, :], rsb[:], ALU.mult)

    # ---- attnT (+bo) + residual ----
    res1 = []
    for et in range(2):
        p = ps.tile([128, 512], f32, tag="mm", name="mma")
        for hh in range(4):
            nc.tensor.matmul(
                p[:],
                wpr[hh // 2][64 * (hh % 2) : 64 * (hh % 2) + 64,
                             OWO + 128 * et : OWO + 128 * et + 128],
                ot4[hh // 2][64 * (hh % 2) : 64 * (hh % 2) + 64, :],
                start=(hh == 0), stop=(hh == 3))
        r = work.tile([128, 512], f32r, tag=f"res{et}", name=f"res1_{et}")
        nc.vector.scalar_tensor_tensor(
            r[:], p[:], colpk[:, 36 + 10 * l + et : 37 + 10 * l + et],
            hg[et], ALU.add, ALU.add)
        res1.append(r)

    # ---- LN1 ----
    r1n = _ln(nc, res1, ones_col, ones_row, colpk, 4 + 8 * l, work, rowp,
              ps1, ps2, "r1n")

    # ---- FFN ----
    f1 = []
    for m in range(2):
        p = ps.tile([128, 512], f32, tag="mm", name="mmf1")
        for c in range(2):
            nc.tensor.matmul(
                p[:], wpr[c][:, OW1 + 128 * m : OW1 + 128 * m + 128],
                r1n[c][:], start=(c == 0), stop=(c == 1))
        t = work.tile([128, 512], f32r, tag=f"qq{2 * m}", name=f"f1_{m}")
        nc.scalar.activation(t[:], p[:], AF.Relu,
                             bias=colpk[:, 38 + 10 * l + m : 39 + 10 * l + m])
        f1.append(t)
    res2 = []
    for et in range(2):
        p = ps.tile([128, 512], f32, tag="mm", name="mmf2")
        for m in range(2):
            nc.tensor.matmul(
                p[:], wpr[m][:, OW2 + 128 * et : OW2 + 128 * et + 128],
                f1[m][:], start=(m == 0), stop=(m == 1))
        r = work.tile([128, 512], f32r, tag=f"res{et}", name=f"res2_{et}")
        nc.vector.scalar_tensor_tensor(
            r[:], p[:], colpk[:, 40 + 10 * l + et : 41 + 10 * l + et],
            r1n[et][:], ALU.add, ALU.add)
        res2.append(r)

    # ---- LN2 -> layer output slice ----
    out = _ln(nc, res2, ones_col, ones_row, colpk, 4 + 8 * l + 4, work, rowp,
              ps1, ps2, "hout")
    for c in range(2):
        nc.vector.tensor_copy(hnew[c][:, 512 * g : 512 * g + 512], out[c][:])


def _ln(nc, xs, ones_col, ones_row, colpk, col_off, work, rowp, ps1, ps2, otag):
    """LayerNorm over the feature axis (partitions, 2 tiles of 128)."""
    pm = ps1.tile([128, 512], f32, tag="lnrow", name="lnrow")
    for c in range(2):
        nc.tensor.matmul(pm[0:1, :], ones_col[:], xs[c][:],
                         start=(c == 0), stop=(c == 1))
    psq = ps1.tile([128, 512], f32, tag="lnrow2", name="lnrow2")
    for c in range(2):
        sq = work.tile([128, 512], f32r, tag="lnscr", name="lnsq")
        nc.vector.tensor_tensor(sq[:], xs[c][:], xs[c][:], ALU.mult)
        nc.tensor.matmul(psq[0:1, :], ones_col[:], sq[:],
                         start=(c == 0), stop=(c == 1))
    mean = rowp.tile([1, 512], f32, tag="lnA", name="lnmean")
    nc.vector.tensor_scalar(mean[:], pm[0:1, :], 1.0 / D, None, ALU.mult)
    var = rowp.tile([1, 512], f32, tag="lnB", name="lnvar")
    nc.vector.tensor_scalar(var[:], psq[0:1, :], 1.0 / D, None, ALU.mult)
    msq = rowp.tile([1, 512], f32, tag="lnC", name="lnmsq")
    nc.vector.tensor_tensor(msq[:], mean[:], mean[:], ALU.mult)
    nc.vector.tensor_tensor(var[:], var[:], msq[:], ALU.subtract)
    lnv = rowp.tile([1, 512], f32, tag="lnC", name="lnv")
    nc.scalar.activation(lnv[:], var[:], AF.Ln, bias=colpk[0:1, 25:26])
    inv = rowp.tile([1, 512], f32r, tag="lnD", name="lninv")
    nc.scalar.activation(inv[:], lnv[:], AF.Exp, scale=colpk[0:1, 27:28])
    meanr = rowp.tile([1, 512], f32r, tag="lnE", name="lnmeanr")
    nc.vector.tensor_copy(meanr[:], mean[:])

    pmb = ps2.tile([128, 512], f32, tag="ps_s", name="lnpmb")
    nc.tensor.matmul(pmb[:], ones_row[0:1, 0:128], meanr[:],
                     start=True, stop=True)
    pib = ps2.tile([128, 512], f32, tag="ps_s", name="lnpib")
    nc.tensor.matmul(pib[:], ones_row[0:1, 0:128], inv[:],
                     start=True, stop=True)
    invb = work.tile([128, 512], f32, tag="lninvb", name="lninvb")
    nc.vector.tensor_copy(invb[:], pib[:])

    out = []
    for c in range(2):
        t1 = work.tile([128, 512], f32, tag="lnscr", name="lnt1")
        nc.vector.tensor_tensor(t1[:], xs[c][:], pmb[:], ALU.subtract)
        t2 = work.tile([128, 512], f32r, tag=otag + str(c),
                       name=otag + str(c))
        nc.vector.scalar_tensor_tensor(
            t2[:], t1[:], colpk[:, col_off + c : col_off + c + 1], invb[:],
            ALU.mult, ALU.mult)
        nc.vector.tensor_scalar(
            t2[:], t2[:], colpk[:, col_off + 2 + c : col_off + 3 + c], None,
            ALU.add)
        out.append(t2)
    return out


# ----------------------------------------------------------------------
# host side
# ----------------------------------------------------------------------

def _prep_host(inputs):
    f = np.float32
    x = np.asarray(inputs["x"], f)
    adj = np.asarray(inputs["adj_fc"])
    spd = np.asarray(inputs["spd_dist"])
    deg = (adj != 0).sum(1)
    deg_rows = np.asarray(inputs["deg_emb"], f)[deg]      # [B, N, D]

    spd_emb = np.asarray(inputs["spd_emb"], f)
    tab = np.empty((101, H), f)
    tab[0] = -1.0
    tab[1:] = spd_emb
    tab_bf = _bf16(np.exp(tab))

    Wqkv = np.asarray(inputs["Wqkv"], f)
    bqkv = np.asarray(inputs["bqkv"], f)

    wpack = np.zeros((D, WCOLS), f)
    rows = np.zeros((1, RCOLS), f)
    rows[0, WCOLS:] = 1.0
    wpack[:, 0:256] = np.asarray(inputs["W_first"], f).T
    rows[0, 0:256] = np.asarray(inputs["b_first"], f)
    for l in range(L):
        o = OFF_L[l]
        wqk = Wqkv[l][:512].T.copy()
        wqk[:, :256] *= SCALE
        wpack[:, o : o + 512] = wqk
        bqk = bqkv[l][:512].copy()
        bqk[:256] *= SCALE
        rows[0, o : o + 512] = bqk
        WvT = Wqkv[l][512:768].T
        for h in range(H):
            wpack[:, o + 512 + 33 * h : o + 512 + 33 * h + 32] = \
                WvT[:, 32 * h : 32 * h + 32]
            rows[0, o + 512 + 33 * h : o + 512 + 33 * h + 32] = \
                bqkv[l][512 + 32 * h : 512 + 32 * h + 32]
            rows[0, o + 512 + 33 * h + 32] = 1.0
        wpack[:, o + 776 : o + 1032] = np.asarray(inputs["Wo"], f)[l].T
        rows[0, o + 776 : o + 1032] = np.asarray(inputs["bo"], f)[l]
        wpack[:, o + 1032 : o + 1288] = np.asarray(inputs["W1"], f)[l].T
        rows[0, o + 1032 : o + 1288] = np.asarray(inputs["b1"], f)[l]
        wpack[:, o + 1288 : o + 1544] = np.asarray(inputs["W2"], f)[l].T
        rows[0, o + 1288 : o + 1544] = np.asarray(inputs["b2"], f)[l]
    wpack[:, OFF_WIN : OFF_WIN + 256] = np.asarray(inputs["W_in"], f).T
    rows[0, OFF_WIN : OFF_WIN + 256] = np.asarray(inputs["b_in"], f)

    cols = np.zeros((128, 52), f)
    for et in range(2):
        sl = slice(128 * et, 128 * et + 128)
        cols[:, 0 + et] = np.asarray(inputs["bn1_g"], f)[sl]
        cols[:, 2 + et] = np.asarray(inputs["bn1_b"], f)[sl]
        cols[:, 20 + et] = np.asarray(inputs["bn2_g"], f)[sl]
        cols[:, 22 + et] = np.asarray(inputs["bn2_b"], f)[sl]
        for l in range(L):
            cols[:, 4 + 8 * l + et] = np.asarray(inputs["ln1_g"], f)[l][sl]
            cols[:, 4 + 8 * l + 2 + et] = np.asarray(inputs["ln1_b"], f)[l][sl]
            cols[:, 4 + 8 * l + 4 + et] = np.asarray(inputs["ln2_g"], f)[l][sl]
            cols[:, 4 + 8 * l + 6 + et] = np.asarray(inputs["ln2_b"], f)[l][sl]
    cols[:, 24] = 1.0
    cols[:, 25] = EPS
    cols[:, 26] = -1.0
    cols[:, 27] = -0.5
    cols[:, 28:30] = np.asarray(inputs["b_first"], f).reshape(2, 128).T
    cols[:, 30:32] = np.asarray(inputs["b_in"], f).reshape(2, 128).T
    for l in range(L):
        base = 32 + 10 * l
        bqk = bqkv[l][:512].copy()
        bqk[:256] *= SCALE
        cols[:, base : base + 4] = bqk.reshape(4, 128).T
        cols[:, base + 4 : base + 6] = \
            np.asarray(inputs["bo"], f)[l].reshape(2, 128).T
        cols[:, base + 6 : base + 8] = \
            np.asarray(inputs["b1"], f)[l].reshape(2, 128).T
        cols[:, base + 8 : base + 10] = \
            np.asarray(inputs["b2"], f)[l].reshape(2, 128).T
    ident = np.eye(128, dtype=f)
    sel4 = np.zeros((128, 128), f)
    for m in range(128):
        sel4[32 * (m // 32), m] = 1.0

    x3 = x.reshape(B, N, DIN)

    in_maps = []
    for k in range(NC):
        gs = [k + 8 * jb for jb in range(G)]
        xTk = np.ascontiguousarray(x3[gs].reshape(NI, DIN).T)
        degTk = np.ascontiguousarray(deg_rows[gs].reshape(NI, D).T)
        biask = np.empty((G * H, N, N), np.uint16)
        for jb in range(G):
            c = k // 4 + 2 * jb
            for h in range(H):
                gsrc = 8 * (k % 4) + h
                biask[jb * 8 + h] = tab_bf[:, c][spd[gsrc].T + 1]
        in_maps.append({
            "xT": xTk, "degT": degTk,
            "biasT": biask.reshape(G * H, 4, 128, N),
            "wpack": wpack, "rows": rows, "cols": cols, "ident": ident,
            "sel4": sel4,
        })
    return in_maps


def kernel(**inputs):
    global LAST_EXEC_NS
    if "nc" not in _CACHE:
        _CACHE["nc"] = build_nc()
    nc = _CACHE["nc"]
    in_maps = _prep_host(inputs)
    trace = bool(int(os.environ.get("GRAPHORMER_TRACE", "0")))
    res = run_bass_kernel_spmd(
        nc, in_maps, core_ids=list(range(NC)), trace=trace)
    LAST_EXEC_NS = res.exec_time_ns

    out = np.empty((B * N, DOUT), np.float32)
    o3 = out.reshape(B, N, DOUT)
    for k in range(NC):
        o3[[k + 8 * jb for jb in range(G)]] = \
            res.results[k]["out"].reshape(G, N, DOUT)
    return out


if __name__ == "__main__":
    import reference
    inputs = {kk: np.asarray(v) for kk, v in reference.setup_inputs().items()}
    t0 = time.time()
    outp = kernel(**inputs)
    t1 = time.time()
    exp = np.asarray(reference.reference(**inputs))
    err = np.abs(outp - exp)
    rel = np.linalg.norm(outp - exp) / np.linalg.norm(exp)
    print(f"wall {t1-t0:.1f}s  absmax {err.max():.3e}  rel(norm) {rel:.3e}")
    print("exec_ns", LAST_EXEC_NS)
